# revision 29
# baseline (speedup 1.0000x reference)
"""Trainium2 Bass kernel for GaborDownsampleBlock.

Computes: conv2d(x, gabor_filters(freq, theta, psi, sigma), stride=2, pad=1)
-> BatchNorm2d (training-mode batch stats) -> LeakyReLU(0.1).

Sharding: data-parallel over the batch dim (4 images per core on 8 cores).
Gabor/BN params are replicated. BN batch statistics are globalized with a
two-stage AllReduce (4-core groups, then cross-pairs).

Per-core layout: each input image is stored in SBUF zero-padded and split by
H-row parity across the 128 partitions: partitions 0-63 hold (i, even padded
rows), partitions 64-127 hold (i, odd padded rows).  Because KS=4/stride=2,
the 4 kh taps pair up two-per-parity, so the conv becomes 8 full K=128
matmuls per PSUM tile (kw in 0..3, kh-pair in 0..1) with float32r inputs.
"""

import math

import numpy as np

import concourse.bacc as bacc
import concourse.mybir as mybir
import concourse.tile as tile
from concourse import bass_utils

N_CORES = 8
B, I, O, H, W = 32, 64, 128, 128, 128
B_LOC = B // N_CORES  # 4
OH = OW = 64
KS = 4
PI = 3.14  # module constant (not math.pi)
LIN = [-1.0, 0.0, 1.0, 2.0]  # linspace(-1, 2, 4)
HP = H // 2 + 1  # 65 padded-row slots per parity
WP = W + 2  # 130 padded cols
N_TILES = B_LOC * 8  # 32 psum tiles of [128, 512] per core
N_GLOBAL = float(B * OH * OW)  # BN sample count per channel

f32 = mybir.dt.float32
f32r = mybir.dt.float32r
AF = mybir.ActivationFunctionType
ALU = mybir.AluOpType


def _body(nc, tc, xd, thetaT, freqT, psiT, sigmaT, gamd, betd, outd, groups2,
          n_global=N_GLOBAL):
    with (
        tc.tile_pool(name="cpool", bufs=1) as cpool,
        tc.tile_pool(name="xpool", bufs=2) as xpool,
        tc.tile_pool(name="ppool", bufs=4, space="PSUM") as ppool,
        tc.tile_pool(name="rpool", bufs=1) as rpool,
        tc.tile_pool(name="spool", bufs=1) as spool,
        tc.tile_pool(name="dram", bufs=1, space="DRAM") as dram,
    ):
        # ---------------- Gabor filter generation (one-time) ----------------
        # Param tiles are [128, O] with partition (g, i) -> param[o, i]
        # (host pre-transposes and stacks the two parity copies).
        th = cpool.tile([128, O], f32)
        nc.sync.dma_start(th[:], thetaT.ap())
        fr = cpool.tile([128, O], f32)
        nc.sync.dma_start(fr[:], freqT.ap())
        ps = cpool.tile([128, O], f32)
        nc.sync.dma_start(ps[:], psiT.ap())
        sg = cpool.tile([128, O], f32)
        nc.sync.dma_start(sg[:], sigmaT.ap())

        phv = cpool.tile([128, 1], f32)
        nc.gpsimd.memset(phv[:], math.pi / 2)
        npv = cpool.tile([128, 1], f32)
        nc.gpsimd.memset(npv[:], -math.pi)
        # cos(t) = sin(pi/2 - t); theta in [0, 7pi/8] keeps the arg in the
        # ScalarE Sin LUT range [-pi, pi]
        ct = cpool.tile([128, O], f32)
        nc.scalar.activation(ct[:], th[:], AF.Sin, bias=phv[:], scale=-1.0)
        st = cpool.tile([128, O], f32)
        nc.scalar.activation(st[:], th[:], AF.Sin)

        sp = cpool.tile([128, O], f32)
        nc.vector.tensor_scalar_add(sp[:], sg[:], 0.001)
        inv_s = cpool.tile([128, O], f32)
        nc.vector.reciprocal(inv_s[:], sp[:])
        c2 = cpool.tile([128, O], f32)
        nc.vector.tensor_mul(c2[:], inv_s[:], inv_s[:])
        nc.vector.tensor_scalar_mul(c2[:], c2[:], -0.5)

        s2 = cpool.tile([128, O], f32)
        nc.vector.tensor_mul(s2[:], sg[:], sg[:])
        rs2 = cpool.tile([128, O], f32)
        nc.vector.reciprocal(rs2[:], s2[:])
        # negated: the range-reduced cos comes out as -cos (see below)
        nrm = cpool.tile([128, O], f32)
        nc.vector.tensor_scalar_mul(nrm[:], rs2[:], -1.0 / (2.0 * PI))

        # y = lin[kh] per-partition vectors; kh = 2*pair + (partition >= 64)
        ya = cpool.tile([128, 1], f32)
        nc.gpsimd.memset(ya[0:64, :], LIN[0])
        nc.gpsimd.memset(ya[64:128, :], LIN[1])
        yb = cpool.tile([128, 1], f32)
        nc.gpsimd.memset(yb[0:64, :], LIN[2])
        nc.gpsimd.memset(yb[64:128, :], LIN[3])

        wbuf = cpool.tile([128, 8 * O], f32)
        wbufr = cpool.tile([128, 8 * O], f32r)
        ri_t = cpool.tile([128, O], mybir.dt.int32)
        rf_t = cpool.tile([128, O], f32)
        gt_t = cpool.tile([128, O], f32)
        rx = cpool.tile([128, O], f32)
        ry = cpool.tile([128, O], f32)
        t1 = cpool.tile([128, O], f32)
        t2 = cpool.tile([128, O], f32)
        ev = cpool.tile([128, O], f32)
        av = cpool.tile([128, O], f32)
        cv = cpool.tile([128, O], f32)
        ctx = cpool.tile([128, O], f32)
        stx = cpool.tile([128, O], f32)
        for kw in range(KS):
            nc.vector.tensor_scalar_mul(ctx[:], ct[:], LIN[kw])
            nc.vector.tensor_scalar_mul(stx[:], st[:], LIN[kw])
            for pair in range(2):
                yv = ya if pair == 0 else yb
                sl = wbuf[:, (kw * 2 + pair) * O : (kw * 2 + pair + 1) * O]
                # rotx = x*cos(t) + y*sin(t);  roty = -x*sin(t) + y*cos(t)
                nc.vector.scalar_tensor_tensor(
                    rx[:], st[:], yv[:], ctx[:], op0=ALU.mult, op1=ALU.add
                )
                nc.vector.scalar_tensor_tensor(
                    ry[:], ct[:], yv[:], stx[:], op0=ALU.mult, op1=ALU.subtract
                )
                nc.vector.tensor_mul(t1[:], rx[:], rx[:])
                nc.vector.tensor_mul(t2[:], ry[:], ry[:])
                nc.vector.tensor_add(t1[:], t1[:], t2[:])
                nc.vector.tensor_mul(t1[:], t1[:], c2[:])
                nc.scalar.activation(ev[:], t1[:], AF.Exp)
                nc.vector.tensor_mul(av[:], fr[:], rx[:])
                nc.vector.tensor_add(av[:], av[:], ps[:])
                # cos(a) = sin(pi/2 - a) = sin(2*pi*u), u = 0.25 - a/2pi.
                # Range-reduce: m = u - floor(u), computed via an f32->i32->f32
                # round trip corrected with (r > u) so it is exact under both
                # truncation (CoreSim) and round-to-nearest (HW). Then
                # sin(2*pi*m - pi) = -sin(2*pi*u); the sign is folded into nrm.
                nc.vector.tensor_scalar(
                    av[:], av[:], -1.0 / (2 * math.pi), 0.25,
                    op0=ALU.mult, op1=ALU.add,
                )
                nc.vector.tensor_copy(ri_t[:], av[:])
                nc.vector.tensor_copy(rf_t[:], ri_t[:])
                nc.vector.tensor_tensor(
                    gt_t[:], rf_t[:], av[:], op=ALU.is_gt
                )
                nc.vector.tensor_sub(rf_t[:], rf_t[:], gt_t[:])
                nc.vector.tensor_sub(av[:], av[:], rf_t[:])
                nc.scalar.activation(
                    cv[:], av[:], AF.Sin, scale=2 * math.pi, bias=npv[:]
                )
                nc.vector.tensor_mul(ev[:], ev[:], cv[:])
                nc.vector.tensor_mul(sl, ev[:], nrm[:])
        # single rounding pass so the matmul consumes fp32r-typed weights
        nc.vector.tensor_copy(wbufr[:], wbuf[:])

        # ---------------- Conv + stats ----------------
        res = rpool.tile([128, N_TILES * 512], f32)
        sums = spool.tile([128, N_TILES], f32)
        sumsqs = spool.tile([128, N_TILES], f32)
        sqscr = spool.tile([128, 512], f32)

        xap = xd.ap()
        for b in range(B_LOC):
            xt = xpool.tile([128, HP * WP], f32r, name="xt")
            xv = xt.rearrange("p (s c) -> p s c", c=WP)
            # zero borders: top pad row (G0 slot 0), bottom pad row (G1 slot
            # HP-1), left/right pad cols
            nc.gpsimd.memset(xv[0:64, 0, :].bitcast(f32), 0.0)
            nc.gpsimd.memset(xv[64:128, HP - 1, :].bitcast(f32), 0.0)
            nc.gpsimd.memset(xv[:, :, 0:1].bitcast(f32), 0.0)
            nc.gpsimd.memset(xv[:, :, WP - 1 : WP].bitcast(f32), 0.0)
            # odd x rows -> G0 slots 1..64; even x rows -> G1 slots 0..63
            nc.sync.dma_start(
                xv[0:64, 1:HP, 1 : W + 1], xap[b, :, 1::2, :].bitcast(f32r)
            )
            nc.sync.dma_start(
                xv[64:128, 0 : HP - 1, 1 : W + 1], xap[b, :, 0::2, :].bitcast(f32r)
            )

            for ohb in range(8):
                pt = ppool.tile([128, 512], f32, name="pt")
                k = 0
                for kw in range(KS):
                    for pair in range(2):
                        s0 = ohb * 8 + pair
                        rhs = xv[:, s0 : s0 + 8, kw : kw + 127 : 2]
                        lhsT = wbufr[
                            :, (kw * 2 + pair) * O : (kw * 2 + pair + 1) * O
                        ]
                        nc.tensor.matmul(
                            pt[:], lhsT, rhs, start=(k == 0), stop=(k == 7)
                        )
                        k += 1
                t = b * 8 + ohb
                nc.scalar.activation(
                    res[:, t * 512 : (t + 1) * 512],
                    pt[:],
                    AF.Copy,
                    accum_out=sums[:, t : t + 1],
                )
                # sum of squares via ACT (tensor_tensor_reduce faults on HW)
                nc.scalar.activation(
                    sqscr[:], pt[:], AF.Square, accum_out=sumsqs[:, t : t + 1]
                )

        # ------- global BN stats (single 8-core AllGather + local sum) ------
        loc = spool.tile([128, 2], f32)
        nc.vector.reduce_sum(loc[:, 0:1], sums[:], axis=mybir.AxisListType.X)
        nc.vector.reduce_sum(loc[:, 1:2], sumsqs[:], axis=mybir.AxisListType.X)

        n_ranks = len(groups2[0][0])
        bin_ = dram.tile([1, 256], f32)
        bout = dram.tile([n_ranks, 256], f32, addr_space="Shared")
        # dram[0, stat*128 + o] = loc[o, stat]
        nc.sync.dma_start(
            bin_[0:1, :].rearrange("a (s o) -> (a o) s", o=128), loc[:]
        )
        nc.gpsimd.collective_compute(
            "AllGather",
            ALU.bypass,
            replica_groups=groups2[0],
            ins=[bin_.opt()],
            outs=[bout.opt()],
        )
        g = spool.tile([128, 2 * n_ranks], f32)
        gv = g.rearrange("o (s r) -> o s r", s=2)
        boutv = bout[:, :].rearrange("r (s o) -> o s r", o=128)
        for s in range(2):
            nc.sync.dma_start(gv[:, s, :], boutv[:, s, :])

        mn = spool.tile([128, 1], f32)
        nc.vector.reduce_sum(mn[:], gv[:, 0, :], axis=mybir.AxisListType.X)
        nc.vector.tensor_scalar_mul(mn[:], mn[:], 1.0 / n_global)
        ex2 = spool.tile([128, 1], f32)
        nc.vector.reduce_sum(ex2[:], gv[:, 1, :], axis=mybir.AxisListType.X)
        nc.vector.tensor_scalar_mul(ex2[:], ex2[:], 1.0 / n_global)
        var = spool.tile([128, 1], f32)
        nc.vector.tensor_mul(var[:], mn[:], mn[:])
        nc.vector.tensor_sub(var[:], ex2[:], var[:])
        nc.vector.tensor_scalar_add(var[:], var[:], 1e-5)
        rin = spool.tile([128, 1], f32)
        nc.vector.reciprocal(rin[:], var[:])
        inv = spool.tile([128, 1], f32)
        nc.scalar.activation(inv[:], rin[:], AF.Sqrt)

        gam = spool.tile([128, 1], f32)
        nc.sync.dma_start(gam[:], gamd.ap())
        bet = spool.tile([128, 1], f32)
        nc.sync.dma_start(bet[:], betd.ap())
        Asc = spool.tile([128, 1], f32)
        nc.vector.tensor_mul(Asc[:], gam[:], inv[:])
        Bsc = spool.tile([128, 1], f32)
        nc.vector.tensor_mul(Bsc[:], Asc[:], mn[:])
        nc.vector.tensor_sub(Bsc[:], bet[:], Bsc[:])

        # ---------------- normalize + LeakyReLU + store ----------------
        oap = outd.ap()
        for b in range(B_LOC):
            for ohb in range(8):
                t = b * 8 + ohb
                sl = res[:, t * 512 : (t + 1) * 512]
                # z = A*v + B, then leaky relu as max(0.1*z, z)
                nc.scalar.activation(
                    sl, sl, AF.Identity, bias=Bsc[:], scale=Asc[:]
                )
                nc.vector.scalar_tensor_tensor(
                    sl, sl, 0.1, sl, op0=ALU.mult, op1=ALU.max
                )
            nc.sync.dma_start(
                oap[b].rearrange("o h w -> o (h w)"),
                res[:, b * 8 * 512 : (b + 1) * 8 * 512],
            )


def build_nc(groups2=None, n_global=N_GLOBAL):
    if groups2 is None:
        groups2 = ([list(range(N_CORES))],)
    nc = bacc.Bacc(
        "TRN2", target_bir_lowering=False, debug=False, num_devices=N_CORES
    )
    xd = nc.dram_tensor("x", [B_LOC, I, H, W], f32, kind="ExternalInput")
    thetaT = nc.dram_tensor("thetaT", [128, O], f32, kind="ExternalInput")
    freqT = nc.dram_tensor("freqT", [128, O], f32, kind="ExternalInput")
    psiT = nc.dram_tensor("psiT", [128, O], f32, kind="ExternalInput")
    sigmaT = nc.dram_tensor("sigmaT", [128, O], f32, kind="ExternalInput")
    gamd = nc.dram_tensor("gamma", [O, 1], f32, kind="ExternalInput")
    betd = nc.dram_tensor("beta", [O, 1], f32, kind="ExternalInput")
    outd = nc.dram_tensor("out", [B_LOC, O, OH, OW], f32, kind="ExternalOutput")
    with tile.TileContext(nc) as tc:
        _body(nc, tc, xd, thetaT, freqT, psiT, sigmaT, gamd, betd, outd, groups2,
              n_global=n_global)
    nc.compile()
    return nc


_NC = None


def _install_ntff_hook():
    """Register the axon NTFF profiling hook if the image's antenv lacks it.

    ``run_bass_kernel_spmd(trace=True)`` under axon imports
    ``antenv.axon_hooks``; this container's antenv has no such module, but
    the ctypes hook implementation ships in ``trn_agent_boot``.
    """
    import sys
    import types

    try:
        import antenv.axon_hooks  # noqa: F401

        return
    except ImportError:
        pass
    try:
        import antenv
        from trn_agent_boot.trn_boot import _ntff_profile_via_ctypes

        hook = _ntff_profile_via_ctypes("/opt/axon/libaxon_pjrt.so")
        if hook is None:
            return
        mod = types.ModuleType("antenv.axon_hooks")
        state = {"hook": hook}
        mod.get_axon_ntff_profile_hook = lambda: state["hook"]
        mod.set_axon_ntff_profile_hook = lambda h: state.update(hook=h)
        sys.modules["antenv.axon_hooks"] = mod
        antenv.axon_hooks = mod
    except Exception:
        pass


def _marshal(x, freq, theta, psi, sigma, gamma, beta):
    """Build the 8 per-core input maps (host-side shard + replicate)."""

    def rep_t(p):
        pt = np.ascontiguousarray(p.T.astype(np.float32))  # [I, O]
        return np.concatenate([pt, pt], axis=0)  # [128, O]

    thetaT = rep_t(theta)
    freqT = rep_t(freq)
    psiT = rep_t(psi)
    sigmaT = rep_t(sigma)
    gam = np.ascontiguousarray(gamma.astype(np.float32).reshape(O, 1))
    bet = np.ascontiguousarray(beta.astype(np.float32).reshape(O, 1))
    in_maps = []
    for c in range(N_CORES):
        in_maps.append(
            {
                "x": np.ascontiguousarray(
                    x[c * B_LOC : (c + 1) * B_LOC].astype(np.float32)
                ),
                "thetaT": thetaT,
                "freqT": freqT,
                "psiT": psiT,
                "sigmaT": sigmaT,
                "gamma": gam,
                "beta": bet,
            }
        )
    return in_maps


def kernel(x, freq, theta, psi, sigma, gamma, beta, _trace=False):
    global _NC
    if _NC is None:
        _NC = build_nc()
    if _trace:
        _install_ntff_hook()
    in_maps = _marshal(x, freq, theta, psi, sigma, gamma, beta)
    res = bass_utils.run_bass_kernel_spmd(
        _NC, in_maps, core_ids=list(range(N_CORES)), trace=_trace
    )
    out = np.concatenate([res.results[c]["out"] for c in range(N_CORES)], axis=0)
    if _trace:
        kernel._last_results = res
    return out
